# revision 2
# baseline (speedup 1.0000x reference)
"""Trainium2 Bass kernel for nn_Interpolator: pilot-to-subcarrier linear
interpolation with learned per-subcarrier weights.

Math: out[b, t] = alpha[t] * Hp[b, right[t]] + beta[t] * Hp[b, left[t]]
where Hp = [H, extrapolated last column]. The op is linear in H, so it
collapses to out = H @ W with W [256, 4096] built on the host from
(pilot_loc, alpha, beta); the extrapolation folds into W's last rows.

Key structure: W's columns repeat — with the module's constant alpha/beta
every stride-16 window of subcarriers shares one column, so W has only
U = 256 *unique* columns. The device computes the compressed product
out_u = H @ W_u (per core: [2048, 256] x [256, U]); the host unshard step
expands columns back to the full [B, 4096, 2] with one gather. That cuts
per-core HBM traffic from ~68 MB to ~4 MB, which is what matters in this
memory-bound regime. If W has no duplicate columns (general alpha/beta),
U = 4096 and the same code degrades to the full product.

The input is packed host-side in bf16, pre-transposed to [pilot, batch]
layout so the contraction dim lands on SBUF partitions directly — no
on-chip transposes. Matmuls accumulate the two 128-pilot halves into
fp32 PSUM; results are copied (vector/scalar engines alternating) into a
bf16 staging tile and stored with wide contiguous DMAs. bf16 rounding of
H and of the output each contribute ~1e-3 relative error, far inside the
2e-2 gate; an optional hi/lo compensation path (use_hlo/use_wlo) exists
for tighter tolerances.

Sharding: data-parallel over the batch dim, 2048 rows per core x 8 cores.
"""

import os
import sys

if os.path.isdir("/opt/trn_rl_repo") and "/opt/trn_rl_repo" not in sys.path:
    sys.path.insert(0, "/opt/trn_rl_repo")

import ml_dtypes
import numpy as np

_BF16 = np.dtype(ml_dtypes.bfloat16)

_B, _P, _NFFT = 16384, 256, 4096
_NC = 8
_BS = _B // _NC          # rows per core
_PT = 128                # partition tile (batch rows per tile)
_NBT = _BS // _PT        # batch tiles per core (16)
_GRP = 4                 # batch tiles per DMA group (512KB-class transfers)

_cache = {}


def _interp_matrix(pilot_loc, alpha, beta):
    """W [256, 4096] f32 such that out = H @ W reproduces the reference."""
    p = pilot_loc.astype(np.float64) - 1.0  # reference: 1-based -> 0-based
    pp = np.concatenate([p, [float(_NFFT - 1)]])
    t = np.arange(_NFFT)
    left = np.clip(np.searchsorted(pp, t, side="right") - 1, 0, _P - 1)
    right = left + 1
    Wf = np.zeros((_P + 1, _NFFT), np.float64)
    Wf[left, t] += beta.astype(np.float64)
    Wf[right, t] += alpha.astype(np.float64)
    # Hp[:, P] = H[:, P-1] + slope * (NFFT-1 - p[-1]),
    # slope = (H[:, P-1] - H[:, P-2]) / (p[-1] - p[-2])  -> linear in H.
    d = (float(_NFFT - 1) - p[-1]) / (p[-1] - p[-2])
    W = Wf[:_P]
    W[_P - 1] += (1.0 + d) * Wf[_P]
    W[_P - 2] += (-d) * Wf[_P]
    return np.ascontiguousarray(W.astype(np.float32))


def _bf16_split(x):
    hi = x.astype(_BF16)
    lo = (x - hi.astype(np.float32)).astype(_BF16)
    return hi, lo


def _build_program(U, narr, use_wlo, grp):
    """Compile the per-core program: out_u = H @ W_u over _NBT batch tiles.

    narr: number of packed H arrays (2 = [r_hi, i_hi]; 4 adds lo parts).
    grp: batch tiles per input/output DMA.
    """
    from contextlib import ExitStack

    import concourse.bacc as bacc
    import concourse.bass as bass
    import concourse.mybir as mybir
    import concourse.tile as tile

    f32 = mybir.dt.float32
    bf16 = mybir.dt.bfloat16

    in_bt = narr * _P            # input cols per batch tile (a, h, r packed)
    out_bt = 2 * U               # output cols per batch tile (r/i x U)
    ngrp = _NBT // grp

    nc = bacc.Bacc("TRN2", target_bir_lowering=False, debug=False,
                   num_devices=_NC)
    # Input: [pilot, batch] packed bf16. Column layout per batch tile bt:
    # col = bt*in_bt + a*256 + h*128 + r  (a: array, h: pilot half, r: row).
    h_in = nc.dram_tensor("hx", [_PT, _NBT * in_bt], bf16,
                          kind="ExternalInput").ap()
    w_in = {"h": nc.dram_tensor("wh", [_P, U], bf16,
                                kind="ExternalInput").ap()}
    if use_wlo:
        w_in["l"] = nc.dram_tensor("wl", [_P, U], bf16,
                                   kind="ExternalInput").ap()
    # Output: row = batch row within tile, col = bt*2U + a*U + u.
    out = nc.dram_tensor("out", [_PT, _NBT * out_bt], bf16,
                         kind="ExternalOutput").ap()

    # terms: (H array index offset within the a-block, W part) pairs that
    # accumulate into each PSUM tile. Arrays pack as [r_hi, i_hi, r_lo?,
    # i_lo?]; per output x in {r, i} the hi part is array x, lo is x+2.
    terms = [(0, "h")]
    if narr == 4:
        terms.append((2, "h"))
    if use_wlo:
        terms.append((0, "l"))

    # U-chunks sized to one PSUM bank of fp32.
    chunks = []
    c0 = 0
    while c0 < U:
        cw = min(512, U - c0)
        chunks.append((c0, cw))
        c0 += cw

    with tile.TileContext(nc) as tc, ExitStack() as ctx:
        const_pool = ctx.enter_context(tc.tile_pool(name="const", bufs=1))
        in_pool = ctx.enter_context(tc.tile_pool(name="inp", bufs=2))
        out_pool = ctx.enter_context(tc.tile_pool(name="outp", bufs=2))
        ps_mm = ctx.enter_context(tc.tile_pool(name="psm", bufs=4,
                                               space="PSUM"))

        # Input/weight loads go on the scalar-engine HWDGE ring so they
        # overlap the output stores on the sync ring (per-ring FIFO).
        in_dma = nc.scalar
        w_sb = {}
        for part, wap in w_in.items():
            for h in (0, 1):
                wt = const_pool.tile([128, U], bf16, tag=f"w{part}{h}")
                in_dma.dma_start(wt[:], wap[128 * h:128 * (h + 1), :])
                w_sb[(part, h)] = wt

        copy_idx = 0
        for g in range(ngrp):
            hx = in_pool.tile([128, grp * in_bt], bf16, tag="hx")
            in_dma.dma_start(
                hx[:], h_in[:, g * grp * in_bt:(g + 1) * grp * in_bt])
            ot = out_pool.tile([128, grp * out_bt], bf16, tag="ot")
            for q in range(grp):
                for a in (0, 1):            # 0 = real, 1 = imag
                    for (c0, cw) in chunks:
                        ps = ps_mm.tile([128, cw], f32, tag="ps")
                        n_mm = 2 * len(terms)
                        j = 0
                        for (ao, wp) in terms:
                            for h in (0, 1):
                                st = hx[:, q * in_bt + (a + ao) * 256
                                        + h * 128:
                                        q * in_bt + (a + ao) * 256
                                        + h * 128 + 128]
                                nc.tensor.matmul(
                                    ps[:], st,
                                    w_sb[(wp, h)][:, c0:c0 + cw],
                                    start=(j == 0),
                                    stop=(j == n_mm - 1),
                                )
                                j += 1
                        dst = ot[:, q * out_bt + a * U + c0:
                                 q * out_bt + a * U + c0 + cw]
                        # ~2:1 vector:scalar split keeps the copy engines
                        # balanced (ACT copies are ~2x slower than DVE).
                        if copy_idx % 3 == 2:
                            nc.scalar.copy(dst, ps[:])
                        else:
                            nc.vector.tensor_copy(dst, ps[:])
                        copy_idx += 1
            nc.sync.dma_start(
                out[:, g * grp * out_bt:(g + 1) * grp * out_bt], ot[:])

    nc.compile()
    return nc


def _get_program(U, narr, use_wlo, grp):
    key = (U, narr, use_wlo, grp)
    prog = _cache.get(key)
    if prog is None:
        prog = _build_program(U, narr, use_wlo, grp)
        _cache[key] = prog
    return prog


def _prepare(H_real, H_imag, pilot_loc, alpha, beta):
    """Build (nc, in_maps, assemble) for the full-input problem."""
    H_real = np.ascontiguousarray(np.asarray(H_real, dtype=np.float32))
    H_imag = np.ascontiguousarray(np.asarray(H_imag, dtype=np.float32))
    pilot_loc = np.asarray(pilot_loc, dtype=np.float32)
    alpha = np.asarray(alpha, dtype=np.float32)
    beta = np.asarray(beta, dtype=np.float32)

    W = _interp_matrix(pilot_loc, alpha, beta)
    # Dedupe identical columns: device computes H @ W_u, host expands.
    Wu, inv = np.unique(W, axis=1, return_inverse=True)
    inv = np.asarray(inv).ravel().astype(np.int64)
    U = Wu.shape[1]

    w_hi, w_lo = _bf16_split(Wu)
    use_wlo = bool(np.any(np.asarray(w_lo) != 0))
    # bf16 H alone keeps norm rel err ~1e-3 (gate 2e-2); the lo path is
    # there only for exotic tolerances.
    narr = 2
    grp = _GRP if U <= 512 else 1
    nc = _get_program(U, narr, use_wlo, grp)

    hr = H_real.astype(_BF16)
    hi = H_imag.astype(_BF16)

    in_maps = []
    for i in range(_NC):
        # [a, bt, r, h, p] -> [p, bt, a, h, r] so a group's columns are one
        # contiguous dram block per partition line.
        h2 = np.stack([hr[i * _BS:(i + 1) * _BS],
                       hi[i * _BS:(i + 1) * _BS]])
        x = h2.reshape(2, _NBT, _PT, 2, 128).transpose(4, 1, 0, 3, 2)
        m = {
            "hx": np.ascontiguousarray(x.reshape(_PT, _NBT * 2 * _P)),
            "wh": w_hi,
        }
        if use_wlo:
            m["wl"] = w_lo
        in_maps.append(m)

    def assemble(results):
        # Per core: [r, bt, a, u] -> [bt*r, a, u]; concat cores; expand u.
        outs = []
        for r in results:
            o = r["out"].reshape(_PT, _NBT, 2, U).transpose(1, 0, 2, 3)
            outs.append(o.reshape(_BS, 2, U))
        ou = np.concatenate(outs, axis=0).astype(np.float32)
        full = np.empty((_B, _NFFT, 2), np.float32)
        full[:, :, 0] = ou[:, 0][:, inv]
        full[:, :, 1] = ou[:, 1][:, inv]
        return full

    return nc, in_maps, assemble


def kernel(H_real, H_imag, pilot_loc, alpha, beta):
    nc, in_maps, assemble = _prepare(H_real, H_imag, pilot_loc, alpha, beta)

    from concourse.bass_utils import run_bass_kernel_spmd

    res = run_bass_kernel_spmd(nc, in_maps, list(range(_NC))).results
    return assemble(res)


# revision 3
# speedup vs baseline: 1.0876x; 1.0876x over previous
"""Trainium2 Bass kernel for nn_Interpolator: pilot-to-subcarrier linear
interpolation with learned per-subcarrier weights.

Math: out[b, t] = alpha[t] * Hp[b, right[t]] + beta[t] * Hp[b, left[t]]
where Hp = [H, extrapolated last column]. The op is linear in H, so it
collapses to out = H @ W with W [256, 4096] built on the host from
(pilot_loc, alpha, beta); the extrapolation folds into W's last rows.

Key structure: W's columns repeat — with the module's constant alpha/beta
every stride-16 window of subcarriers shares one column, so W has only
U = 256 *unique* columns. The device computes the compressed product
out_u = H @ W_u (per core: [2048, 256] x [256, U]); the host unshard step
expands columns back to the full [B, 4096, 2] with one gather. That cuts
per-core HBM traffic from ~68 MB to ~4 MB, which is what matters in this
memory-bound regime. If W has no duplicate columns (general alpha/beta),
U = 4096 and the same code degrades to the full product.

Schedule notes (from NTFF traces):
- Input is packed host-side in bf16, pre-transposed to [pilot, batch]
  layout so the contraction dim lands on SBUF partitions directly — no
  on-chip transposes. W_u rides in the head of the same dram tensor so
  the first DMA delivers weights + first batch group together.
- Input/output DMAs alternate between the sync and scalar HWDGE rings:
  each ring serializes its DMAs' ~1.5-2us completion receipts, so one
  ring alone roughly halves effective bandwidth. The first load goes on
  the sync ring because the scalar ring is blocked ~8.5us at startup
  behind ACT_TABLE_LOAD.
- A warmup burst of matmuls on a zeroed tile keeps the PE busy from
  program start so HAM un-throttles the PE clock (1.2 -> 2.4 GHz)
  before the real matmuls arrive; without it every matmul runs cold.
- PSUM->SBUF copies alternate ~3:2 vector:scalar to balance the two
  engines that can read PSUM.

bf16 rounding of H and of the output each contribute ~1e-3 relative
error, far inside the 2e-2 gate; an optional lo-compensation path
(narr=4 / use_wlo) exists for tighter tolerances.

Sharding: data-parallel over the batch dim, 2048 rows per core x 8 cores.
"""

import os
import sys

if os.path.isdir("/opt/trn_rl_repo") and "/opt/trn_rl_repo" not in sys.path:
    sys.path.insert(0, "/opt/trn_rl_repo")

import ml_dtypes
import numpy as np

_BF16 = np.dtype(ml_dtypes.bfloat16)

_B, _P, _NFFT = 16384, 256, 4096
_NC = 8
_BS = _B // _NC          # rows per core
_PT = 128                # partition tile (batch rows per tile)
_NBT = _BS // _PT        # batch tiles per core (16)
_GRP = 4                 # batch tiles per DMA group (512KB-class transfers)
_NWARM = 80              # PE warmup matmuls (N=128): ~4.3us cold + tail warm

_cache = {}


def _interp_matrix(pilot_loc, alpha, beta):
    """W [256, 4096] f32 such that out = H @ W reproduces the reference."""
    p = pilot_loc.astype(np.float64) - 1.0  # reference: 1-based -> 0-based
    pp = np.concatenate([p, [float(_NFFT - 1)]])
    t = np.arange(_NFFT)
    left = np.clip(np.searchsorted(pp, t, side="right") - 1, 0, _P - 1)
    right = left + 1
    Wf = np.zeros((_P + 1, _NFFT), np.float64)
    Wf[left, t] += beta.astype(np.float64)
    Wf[right, t] += alpha.astype(np.float64)
    # Hp[:, P] = H[:, P-1] + slope * (NFFT-1 - p[-1]),
    # slope = (H[:, P-1] - H[:, P-2]) / (p[-1] - p[-2])  -> linear in H.
    d = (float(_NFFT - 1) - p[-1]) / (p[-1] - p[-2])
    W = Wf[:_P]
    W[_P - 1] += (1.0 + d) * Wf[_P]
    W[_P - 2] += (-d) * Wf[_P]
    return np.ascontiguousarray(W.astype(np.float32))


def _bf16_split(x):
    hi = x.astype(_BF16)
    lo = (x - hi.astype(np.float32)).astype(_BF16)
    return hi, lo


def _build_program(U, narr, use_wlo, grp):
    """Compile the per-core program: out_u = H @ W_u over _NBT batch tiles.

    narr: number of packed H arrays (2 = [r_hi, i_hi]; 4 adds lo parts).
    grp: batch tiles per input/output DMA group.
    """
    from contextlib import ExitStack

    import concourse.bacc as bacc
    import concourse.mybir as mybir
    import concourse.tile as tile

    f32 = mybir.dt.float32
    bf16 = mybir.dt.bfloat16

    in_bt = narr * _P            # input cols per batch tile (a, h, r packed)
    out_bt = 2 * U               # output cols per batch tile (r/i x U)
    ngrp = _NBT // grp
    nw = 2 if use_wlo else 1
    woff = nw * 2 * U            # W block cols at the head of the input

    nc = bacc.Bacc("TRN2", target_bir_lowering=False, debug=False,
                   num_devices=_NC)
    # Input: [pilot, batch] packed bf16. Head: W blocks, col = wp*2U + h*U
    # + u. Body: per batch tile bt, col = woff + bt*in_bt + a*256 + h*128
    # + r  (a: array, h: pilot half, r: row within tile).
    h_in = nc.dram_tensor("hx", [_PT, woff + _NBT * in_bt], bf16,
                          kind="ExternalInput").ap()
    # Output: row = batch row within tile, col = bt*2U + a*U + u.
    out = nc.dram_tensor("out", [_PT, _NBT * out_bt], bf16,
                         kind="ExternalOutput").ap()

    # terms: (H array offset, W part index) accumulated into each PSUM tile.
    terms = [(0, 0)]
    if narr == 4:
        terms.append((2, 0))
    if use_wlo:
        terms.append((0, 1))

    # U-chunks sized to one PSUM bank of fp32.
    chunks = []
    c0 = 0
    while c0 < U:
        cw = min(512, U - c0)
        chunks.append((c0, cw))
        c0 += cw

    with tile.TileContext(nc) as tc, ExitStack() as ctx:
        const_pool = ctx.enter_context(tc.tile_pool(name="const", bufs=1))
        g0_pool = ctx.enter_context(tc.tile_pool(name="g0", bufs=1))
        in_pool = ctx.enter_context(tc.tile_pool(name="inp", bufs=2))
        out_pool = ctx.enter_context(tc.tile_pool(name="outp", bufs=2))
        ps_warm = ctx.enter_context(tc.tile_pool(name="psw", bufs=1,
                                                 space="PSUM"))
        ps_mm = ctx.enter_context(tc.tile_pool(name="psm", bufs=4,
                                               space="PSUM"))

        # PE warmup: keep TensorE busy from program start so HAM raises
        # the PE clock before the first real matmul.
        zt = const_pool.tile([128, 128], bf16, tag="zt")
        nc.vector.memset(zt[:], 0.0)
        wps = ps_warm.tile([128, 128], f32, tag="wps")
        for _ in range(_NWARM):
            nc.tensor.matmul(wps[:], zt[:], zt[:], start=True, stop=True)

        rings = (nc.sync, nc.scalar)

        # First DMA: W blocks + group 0, on the sync ring (scalar ring is
        # blocked at startup behind ACT_TABLE_LOAD).
        t0 = g0_pool.tile([128, woff + grp * in_bt], bf16, tag="t0")
        rings[0].dma_start(t0[:], h_in[:, 0:woff + grp * in_bt])

        def w_slice(wp, h, c0, cw):
            return t0[:, wp * 2 * U + h * U + c0:wp * 2 * U + h * U + c0 + cw]

        copy_idx = 0
        for g in range(ngrp):
            if g == 0:
                hx = t0
                base = woff
            else:
                hx = in_pool.tile([128, grp * in_bt], bf16, tag="hx")
                rings[g % 2].dma_start(
                    hx[:], h_in[:, woff + g * grp * in_bt:
                                 woff + (g + 1) * grp * in_bt])
                base = 0
            ot = out_pool.tile([128, grp * out_bt], bf16, tag="ot")
            for q in range(grp):
                for a in (0, 1):            # 0 = real, 1 = imag
                    for (c0, cw) in chunks:
                        ps = ps_mm.tile([128, cw], f32, tag="ps")
                        n_mm = 2 * len(terms)
                        j = 0
                        for (ao, wp) in terms:
                            for h in (0, 1):
                                st = hx[:, base + q * in_bt + (a + ao) * 256
                                        + h * 128:
                                        base + q * in_bt + (a + ao) * 256
                                        + h * 128 + 128]
                                nc.tensor.matmul(
                                    ps[:], st, w_slice(wp, h, c0, cw),
                                    start=(j == 0),
                                    stop=(j == n_mm - 1),
                                )
                                j += 1
                        dst = ot[:, q * out_bt + a * U + c0:
                                 q * out_bt + a * U + c0 + cw]
                        # ~3:2 vector:scalar keeps the PSUM-copy engines
                        # balanced (ACT is a bit slower than DVE).
                        if copy_idx % 5 in (2, 4):
                            nc.scalar.copy(dst, ps[:])
                        else:
                            nc.vector.tensor_copy(dst, ps[:])
                        copy_idx += 1
            # Output rides the opposite ring from this group's input.
            rings[(g + 1) % 2].dma_start(
                out[:, g * grp * out_bt:(g + 1) * grp * out_bt], ot[:])

    nc.compile()
    return nc


def _get_program(U, narr, use_wlo, grp):
    key = (U, narr, use_wlo, grp)
    prog = _cache.get(key)
    if prog is None:
        prog = _build_program(U, narr, use_wlo, grp)
        _cache[key] = prog
    return prog


def _prepare(H_real, H_imag, pilot_loc, alpha, beta):
    """Build (nc, in_maps, assemble) for the full-input problem."""
    H_real = np.ascontiguousarray(np.asarray(H_real, dtype=np.float32))
    H_imag = np.ascontiguousarray(np.asarray(H_imag, dtype=np.float32))
    pilot_loc = np.asarray(pilot_loc, dtype=np.float32)
    alpha = np.asarray(alpha, dtype=np.float32)
    beta = np.asarray(beta, dtype=np.float32)

    W = _interp_matrix(pilot_loc, alpha, beta)
    # Dedupe identical columns: device computes H @ W_u, host expands.
    Wu, inv = np.unique(W, axis=1, return_inverse=True)
    inv = np.asarray(inv).ravel().astype(np.int64)
    U = Wu.shape[1]

    w_hi, w_lo = _bf16_split(Wu)
    use_wlo = bool(np.any(np.asarray(w_lo) != 0))
    # bf16 H alone keeps norm rel err ~1e-3 (gate 2e-2); the lo path is
    # there only for exotic tolerances.
    narr = 2
    grp = _GRP if U <= 512 else 1
    nc = _get_program(U, narr, use_wlo, grp)

    # W head block: [p, wp*2U + h*U + u] = Wpart[h*128 + p, u].
    wparts = [w_hi] + ([w_lo] if use_wlo else [])
    wblk = np.concatenate(
        [np.asarray(wp).reshape(2, 128, U).transpose(1, 0, 2).reshape(128,
                                                                      2 * U)
         for wp in wparts], axis=1)

    hr = H_real.astype(_BF16)
    hi = H_imag.astype(_BF16)

    in_maps = []
    for i in range(_NC):
        # [a, bt, r, h, p] -> [p, bt, a, h, r] so a group's columns are one
        # contiguous dram block per partition line.
        h2 = np.stack([hr[i * _BS:(i + 1) * _BS],
                       hi[i * _BS:(i + 1) * _BS]])
        x = h2.reshape(2, _NBT, _PT, 2, 128).transpose(4, 1, 0, 3, 2)
        m = {
            "hx": np.ascontiguousarray(np.concatenate(
                [wblk, x.reshape(_PT, _NBT * 2 * _P)], axis=1)),
        }
        in_maps.append(m)

    def assemble(results):
        # Per core: [r, bt, a, u] -> [bt*r, a, u]; concat cores; expand u.
        outs = []
        for r in results:
            o = r["out"].reshape(_PT, _NBT, 2, U).transpose(1, 0, 2, 3)
            outs.append(o.reshape(_BS, 2, U))
        ou = np.concatenate(outs, axis=0).astype(np.float32)
        full = np.empty((_B, _NFFT, 2), np.float32)
        full[:, :, 0] = ou[:, 0][:, inv]
        full[:, :, 1] = ou[:, 1][:, inv]
        return full

    return nc, in_maps, assemble


def kernel(H_real, H_imag, pilot_loc, alpha, beta):
    nc, in_maps, assemble = _prepare(H_real, H_imag, pilot_loc, alpha, beta)

    from concourse.bass_utils import run_bass_kernel_spmd

    res = run_bass_kernel_spmd(nc, in_maps, list(range(_NC))).results
    return assemble(res)


# revision 4
# speedup vs baseline: 1.3336x; 1.2261x over previous
"""Trainium2 Bass kernel for nn_Interpolator: pilot-to-subcarrier linear
interpolation with learned per-subcarrier weights.

Math: out[b, t] = alpha[t] * Hp[b, right[t]] + beta[t] * Hp[b, left[t]]
where Hp = [H, extrapolated last column]. The op is linear in H, so it
collapses to out = H @ W with W [256, 4096] built on the host from
(pilot_loc, alpha, beta); the extrapolation folds into W's last rows.

Key structure: W's columns repeat — with the module's constant alpha/beta
every stride-16 window of subcarriers shares one column, so W has only
U = 256 *unique* columns. The device computes the compressed product
out_u = H @ W_u (per core: [2048, 256] x [256, U]); the host unshard step
expands columns back to the full [B, 4096, 2] with one gather. That cuts
per-core HBM traffic from ~68 MB to ~4 MB, which is what matters in this
memory-bound regime. If W has no duplicate columns (general alpha/beta),
U = 4096 and the same code degrades to the full product.

Schedule notes (from NTFF traces):
- Input is packed host-side in bf16, pre-transposed to [pilot, batch]
  layout so the contraction dim lands on SBUF partitions directly — no
  on-chip transposes. W_u rides in the head of the same dram tensor so
  the first DMA delivers weights + first batch group together.
- Input/output DMAs alternate between the sync and scalar HWDGE rings:
  each ring serializes its DMAs' ~1.5-2us completion receipts, so one
  ring alone roughly halves effective bandwidth. The first load goes on
  the sync ring because the scalar ring is blocked ~8.5us at startup
  behind ACT_TABLE_LOAD.
- A warmup burst of matmuls on a zeroed tile keeps the PE busy from
  program start so HAM un-throttles the PE clock (1.2 -> 2.4 GHz)
  before the real matmuls arrive; without it every matmul runs cold.
- PSUM->SBUF copies alternate ~3:2 vector:scalar to balance the two
  engines that can read PSUM.

bf16 rounding of H and of the output each contribute ~1e-3 relative
error, far inside the 2e-2 gate; an optional lo-compensation path
(narr=4 / use_wlo) exists for tighter tolerances.

Sharding: data-parallel over the batch dim, 2048 rows per core x 8 cores.
"""

import os
import sys

if os.path.isdir("/opt/trn_rl_repo") and "/opt/trn_rl_repo" not in sys.path:
    sys.path.insert(0, "/opt/trn_rl_repo")

import ml_dtypes
import numpy as np

_BF16 = np.dtype(ml_dtypes.bfloat16)

_B, _P, _NFFT = 16384, 256, 4096
_NC = 8
_BS = _B // _NC          # rows per core
_PT = 128                # partition tile (batch rows per tile)
_NBT = _BS // _PT        # batch tiles per core (16)
_GRP = 4                 # batch tiles per DMA group (512KB-class transfers)
_NWARM = 42              # PE warmup matmuls (N=128): bridge init->first real MM

_cache = {}


def _interp_matrix(pilot_loc, alpha, beta):
    """W [256, 4096] f32 such that out = H @ W reproduces the reference."""
    p = pilot_loc.astype(np.float64) - 1.0  # reference: 1-based -> 0-based
    pp = np.concatenate([p, [float(_NFFT - 1)]])
    t = np.arange(_NFFT)
    left = np.clip(np.searchsorted(pp, t, side="right") - 1, 0, _P - 1)
    right = left + 1
    Wf = np.zeros((_P + 1, _NFFT), np.float64)
    Wf[left, t] += beta.astype(np.float64)
    Wf[right, t] += alpha.astype(np.float64)
    # Hp[:, P] = H[:, P-1] + slope * (NFFT-1 - p[-1]),
    # slope = (H[:, P-1] - H[:, P-2]) / (p[-1] - p[-2])  -> linear in H.
    d = (float(_NFFT - 1) - p[-1]) / (p[-1] - p[-2])
    W = Wf[:_P]
    W[_P - 1] += (1.0 + d) * Wf[_P]
    W[_P - 2] += (-d) * Wf[_P]
    return np.ascontiguousarray(W.astype(np.float32))


def _bf16_split(x):
    hi = x.astype(_BF16)
    lo = (x - hi.astype(np.float32)).astype(_BF16)
    return hi, lo


def _build_program(U, narr, use_wlo, grp):
    """Compile the per-core program: out_u = H @ W_u over _NBT batch tiles.

    narr: number of packed H arrays (2 = [r_hi, i_hi]; 4 adds lo parts).
    grp: batch tiles per input/output DMA group.
    """
    from contextlib import ExitStack

    import concourse.bacc as bacc
    import concourse.mybir as mybir
    import concourse.tile as tile

    f32 = mybir.dt.float32
    bf16 = mybir.dt.bfloat16

    in_bt = narr * _P            # input cols per batch tile (a, h, r packed)
    out_bt = 2 * U               # output cols per batch tile (r/i x U)
    ngrp = _NBT // grp
    nw = 2 if use_wlo else 1
    woff = nw * 2 * U            # W block cols at the head of the input

    nc = bacc.Bacc("TRN2", target_bir_lowering=False, debug=False,
                   num_devices=_NC)
    # Input: [pilot, batch] packed bf16. Head: W blocks, col = wp*2U + h*U
    # + u. Body: per batch tile bt, col = woff + bt*in_bt + a*256 + h*128
    # + r  (a: array, h: pilot half, r: row within tile).
    h_in = nc.dram_tensor("hx", [_PT, woff + _NBT * in_bt], bf16,
                          kind="ExternalInput").ap()
    # Output: row = batch row within tile, col = bt*2U + a*U + u.
    out = nc.dram_tensor("out", [_PT, _NBT * out_bt], bf16,
                         kind="ExternalOutput").ap()

    # terms: (H array offset, W part index) accumulated into each PSUM tile.
    terms = [(0, 0)]
    if narr == 4:
        terms.append((2, 0))
    if use_wlo:
        terms.append((0, 1))

    # U-chunks sized to one PSUM bank of fp32.
    chunks = []
    c0 = 0
    while c0 < U:
        cw = min(512, U - c0)
        chunks.append((c0, cw))
        c0 += cw

    with tile.TileContext(nc) as tc, ExitStack() as ctx:
        const_pool = ctx.enter_context(tc.tile_pool(name="const", bufs=1))
        g0_pool = ctx.enter_context(tc.tile_pool(name="g0", bufs=1))
        in_pool = ctx.enter_context(tc.tile_pool(name="inp", bufs=3))
        out_pool = ctx.enter_context(tc.tile_pool(name="outp", bufs=2))
        ps_warm = ctx.enter_context(tc.tile_pool(name="psw", bufs=1,
                                                 space="PSUM"))
        ps_mm = ctx.enter_context(tc.tile_pool(name="psm", bufs=4,
                                               space="PSUM"))

        # PE warmup: keep TensorE busy from program start so HAM raises
        # the PE clock before the first real matmul.
        zt = const_pool.tile([128, 128], bf16, tag="zt")
        nc.vector.memset(zt[:], 0.0)
        wps = ps_warm.tile([128, 128], f32, tag="wps")
        for _ in range(_NWARM):
            nc.tensor.matmul(wps[:], zt[:], zt[:], start=True, stop=True)

        rings = (nc.sync, nc.scalar)

        # First DMA: W blocks + group 0, on the sync ring (scalar ring is
        # blocked at startup behind ACT_TABLE_LOAD).
        t0 = g0_pool.tile([128, woff + grp * in_bt], bf16, tag="t0")
        rings[0].dma_start(t0[:], h_in[:, 0:woff + grp * in_bt])

        def w_slice(wp, h, c0, cw):
            return t0[:, wp * 2 * U + h * U + c0:wp * 2 * U + h * U + c0 + cw]

        copy_idx = 0
        for g in range(ngrp):
            if g == 0:
                hx = t0
                base = woff
            else:
                hx = in_pool.tile([128, grp * in_bt], bf16, tag="hx")
                rings[g % 2].dma_start(
                    hx[:], h_in[:, woff + g * grp * in_bt:
                                 woff + (g + 1) * grp * in_bt])
                base = 0
            ot = out_pool.tile([128, grp * out_bt], bf16, tag="ot")
            for q in range(grp):
                for a in (0, 1):            # 0 = real, 1 = imag
                    for (c0, cw) in chunks:
                        ps = ps_mm.tile([128, cw], f32, tag="ps")
                        n_mm = 2 * len(terms)
                        j = 0
                        for (ao, wp) in terms:
                            for h in (0, 1):
                                st = hx[:, base + q * in_bt + (a + ao) * 256
                                        + h * 128:
                                        base + q * in_bt + (a + ao) * 256
                                        + h * 128 + 128]
                                nc.tensor.matmul(
                                    ps[:], st, w_slice(wp, h, c0, cw),
                                    start=(j == 0),
                                    stop=(j == n_mm - 1),
                                )
                                j += 1
                        dst = ot[:, q * out_bt + a * U + c0:
                                 q * out_bt + a * U + c0 + cw]
                        # ~3:2 vector:scalar keeps the PSUM-copy engines
                        # balanced (ACT is a bit slower than DVE).
                        if copy_idx % 5 in (2, 4):
                            nc.scalar.copy(dst, ps[:])
                        else:
                            nc.vector.tensor_copy(dst, ps[:])
                        copy_idx += 1
            # Output rides the opposite ring from this group's input.
            rings[(g + 1) % 2].dma_start(
                out[:, g * grp * out_bt:(g + 1) * grp * out_bt], ot[:])

    nc.compile()
    return nc


def _get_program(U, narr, use_wlo, grp):
    key = (U, narr, use_wlo, grp)
    prog = _cache.get(key)
    if prog is None:
        prog = _build_program(U, narr, use_wlo, grp)
        _cache[key] = prog
    return prog


def _prepare(H_real, H_imag, pilot_loc, alpha, beta):
    """Build (nc, in_maps, assemble) for the full-input problem."""
    H_real = np.ascontiguousarray(np.asarray(H_real, dtype=np.float32))
    H_imag = np.ascontiguousarray(np.asarray(H_imag, dtype=np.float32))
    pilot_loc = np.asarray(pilot_loc, dtype=np.float32)
    alpha = np.asarray(alpha, dtype=np.float32)
    beta = np.asarray(beta, dtype=np.float32)

    W = _interp_matrix(pilot_loc, alpha, beta)
    # Dedupe identical columns: device computes H @ W_u, host expands.
    Wu, inv = np.unique(W, axis=1, return_inverse=True)
    inv = np.asarray(inv).ravel().astype(np.int64)
    U = Wu.shape[1]

    w_hi, w_lo = _bf16_split(Wu)
    use_wlo = bool(np.any(np.asarray(w_lo) != 0))
    # bf16 H alone keeps norm rel err ~1e-3 (gate 2e-2); the lo path is
    # there only for exotic tolerances.
    narr = 2
    grp = _GRP if U <= 512 else 1
    nc = _get_program(U, narr, use_wlo, grp)

    # W head block: [p, wp*2U + h*U + u] = Wpart[h*128 + p, u].
    wparts = [w_hi] + ([w_lo] if use_wlo else [])
    wblk = np.concatenate(
        [np.asarray(wp).reshape(2, 128, U).transpose(1, 0, 2).reshape(128,
                                                                      2 * U)
         for wp in wparts], axis=1)

    hr = H_real.astype(_BF16)
    hi = H_imag.astype(_BF16)

    in_maps = []
    for i in range(_NC):
        # [a, bt, r, h, p] -> [p, bt, a, h, r] so a group's columns are one
        # contiguous dram block per partition line.
        h2 = np.stack([hr[i * _BS:(i + 1) * _BS],
                       hi[i * _BS:(i + 1) * _BS]])
        x = h2.reshape(2, _NBT, _PT, 2, 128).transpose(4, 1, 0, 3, 2)
        m = {
            "hx": np.ascontiguousarray(np.concatenate(
                [wblk, x.reshape(_PT, _NBT * 2 * _P)], axis=1)),
        }
        in_maps.append(m)

    def assemble(results):
        # Per core: [r, bt, a, u] -> [bt*r, a, u]; concat cores; expand u.
        outs = []
        for r in results:
            o = r["out"].reshape(_PT, _NBT, 2, U).transpose(1, 0, 2, 3)
            outs.append(o.reshape(_BS, 2, U))
        ou = np.concatenate(outs, axis=0).astype(np.float32)
        full = np.empty((_B, _NFFT, 2), np.float32)
        full[:, :, 0] = ou[:, 0][:, inv]
        full[:, :, 1] = ou[:, 1][:, inv]
        return full

    return nc, in_maps, assemble


def kernel(H_real, H_imag, pilot_loc, alpha, beta):
    nc, in_maps, assemble = _prepare(H_real, H_imag, pilot_loc, alpha, beta)

    from concourse.bass_utils import run_bass_kernel_spmd

    res = run_bass_kernel_spmd(nc, in_maps, list(range(_NC))).results
    return assemble(res)


# revision 5
# speedup vs baseline: 1.3466x; 1.0098x over previous
"""Trainium2 Bass kernel for nn_Interpolator: pilot-to-subcarrier linear
interpolation with learned per-subcarrier weights.

Math: out[b, t] = alpha[t] * Hp[b, right[t]] + beta[t] * Hp[b, left[t]]
where Hp = [H, extrapolated last column]. The op is linear in H, so it
collapses to out = H @ W with W [256, 4096] built on the host from
(pilot_loc, alpha, beta); the extrapolation folds into W's last rows.

Key structure: W's columns repeat — with the module's constant alpha/beta
every stride-16 window of subcarriers shares one column, so W has only
U = 256 *unique* columns. The device computes the compressed product
out_u = H @ W_u (per core: [2048, 256] x [256, U]); the host unshard step
expands columns back to the full [B, 4096, 2] with one gather. That cuts
per-core HBM traffic from ~68 MB to ~4 MB, which is what matters in this
memory-bound regime. If W has no duplicate columns (general alpha/beta),
U = 4096 and the same code degrades to the full product.

Schedule notes (from NTFF traces):
- Input is packed host-side in bf16, pre-transposed to [pilot, batch]
  layout so the contraction dim lands on SBUF partitions directly — no
  on-chip transposes. W_u rides in the head of the same dram tensor so
  the first DMA delivers weights + first batch group together.
- Input/output DMAs alternate between the sync and scalar HWDGE rings:
  each ring serializes its DMAs' ~1.5-2us completion receipts, so one
  ring alone roughly halves effective bandwidth. The first load goes on
  the sync ring because the scalar ring is blocked ~8.5us at startup
  behind ACT_TABLE_LOAD.
- A warmup burst of matmuls on a zeroed tile keeps the PE busy from
  program start so HAM un-throttles the PE clock (1.2 -> 2.4 GHz)
  before the real matmuls arrive; without it every matmul runs cold.
- PSUM->SBUF copies alternate ~3:2 vector:scalar to balance the two
  engines that can read PSUM.

bf16 rounding of H and of the output each contribute ~1e-3 relative
error, far inside the 2e-2 gate; an optional lo-compensation path
(narr=4 / use_wlo) exists for tighter tolerances.

Sharding: data-parallel over the batch dim, 2048 rows per core x 8 cores.
"""

import os
import sys

if os.path.isdir("/opt/trn_rl_repo") and "/opt/trn_rl_repo" not in sys.path:
    sys.path.insert(0, "/opt/trn_rl_repo")

import ml_dtypes
import numpy as np

_BF16 = np.dtype(ml_dtypes.bfloat16)

_B, _P, _NFFT = 16384, 256, 4096
_NC = 8
_BS = _B // _NC          # rows per core
_PT = 128                # partition tile (batch rows per tile)
_NBT = _BS // _PT        # batch tiles per core (16)
_GRP = 4                 # batch tiles per DMA group (512KB-class transfers)
_NWARM = 42              # PE warmup matmuls (N=128): bridge init->first real MM

_cache = {}


def _interp_matrix(pilot_loc, alpha, beta):
    """W [256, 4096] f32 such that out = H @ W reproduces the reference."""
    p = pilot_loc.astype(np.float64) - 1.0  # reference: 1-based -> 0-based
    pp = np.concatenate([p, [float(_NFFT - 1)]])
    t = np.arange(_NFFT)
    left = np.clip(np.searchsorted(pp, t, side="right") - 1, 0, _P - 1)
    right = left + 1
    Wf = np.zeros((_P + 1, _NFFT), np.float64)
    Wf[left, t] += beta.astype(np.float64)
    Wf[right, t] += alpha.astype(np.float64)
    # Hp[:, P] = H[:, P-1] + slope * (NFFT-1 - p[-1]),
    # slope = (H[:, P-1] - H[:, P-2]) / (p[-1] - p[-2])  -> linear in H.
    d = (float(_NFFT - 1) - p[-1]) / (p[-1] - p[-2])
    W = Wf[:_P]
    W[_P - 1] += (1.0 + d) * Wf[_P]
    W[_P - 2] += (-d) * Wf[_P]
    return np.ascontiguousarray(W.astype(np.float32))


def _bf16_split(x):
    hi = x.astype(_BF16)
    lo = (x - hi.astype(np.float32)).astype(_BF16)
    return hi, lo


def _build_program(U, narr, use_wlo, grp):
    """Compile the per-core program: out_u = H @ W_u over _NBT batch tiles.

    narr: number of packed H arrays (2 = [r_hi, i_hi]; 4 adds lo parts).
    grp: batch tiles per input/output DMA group.
    """
    from contextlib import ExitStack

    import concourse.bacc as bacc
    import concourse.mybir as mybir
    import concourse.tile as tile

    f32 = mybir.dt.float32
    bf16 = mybir.dt.bfloat16

    in_bt = narr * _P            # input cols per batch tile (a, h, r packed)
    out_bt = 2 * U               # output cols per batch tile (r/i x U)
    ngrp = _NBT // grp
    nw = 2 if use_wlo else 1
    woff = nw * 2 * U            # W block cols at the head of the input

    nc = bacc.Bacc("TRN2", target_bir_lowering=False, debug=False,
                   num_devices=_NC)
    # Input: [pilot, batch] packed bf16. Head: W blocks, col = wp*2U + h*U
    # + u. Body: per batch tile bt, col = woff + bt*in_bt + a*256 + h*128
    # + r  (a: array, h: pilot half, r: row within tile).
    h_in = nc.dram_tensor("hx", [_PT, woff + _NBT * in_bt], bf16,
                          kind="ExternalInput").ap()
    # Output: row = batch row within tile, col = bt*2U + a*U + u.
    out = nc.dram_tensor("out", [_PT, _NBT * out_bt], bf16,
                         kind="ExternalOutput").ap()

    # terms: (H array offset, W part index) accumulated into each PSUM tile.
    terms = [(0, 0)]
    if narr == 4:
        terms.append((2, 0))
    if use_wlo:
        terms.append((0, 1))

    # U-chunks sized to one PSUM bank of fp32.
    chunks = []
    c0 = 0
    while c0 < U:
        cw = min(512, U - c0)
        chunks.append((c0, cw))
        c0 += cw

    with tile.TileContext(nc) as tc, ExitStack() as ctx:
        const_pool = ctx.enter_context(tc.tile_pool(name="const", bufs=1))
        g0_pool = ctx.enter_context(tc.tile_pool(name="g0", bufs=1))
        in_pool = ctx.enter_context(tc.tile_pool(name="inp", bufs=3))
        out_pool = ctx.enter_context(tc.tile_pool(name="outp", bufs=4))
        ps_warm = ctx.enter_context(tc.tile_pool(name="psw", bufs=1,
                                                 space="PSUM"))
        ps_mm = ctx.enter_context(tc.tile_pool(name="psm", bufs=6,
                                               space="PSUM"))

        # PE warmup: keep TensorE busy from program start so HAM raises
        # the PE clock before the first real matmul.
        zt = const_pool.tile([128, 128], bf16, tag="zt")
        nc.vector.memset(zt[:], 0.0)
        wps = ps_warm.tile([128, 128], f32, tag="wps")
        for _ in range(_NWARM):
            nc.tensor.matmul(wps[:], zt[:], zt[:], start=True, stop=True)

        rings = (nc.sync, nc.scalar)

        # First DMA: W blocks + group 0, on the sync ring (scalar ring is
        # blocked at startup behind ACT_TABLE_LOAD).
        t0 = g0_pool.tile([128, woff + grp * in_bt], bf16, tag="t0")
        rings[0].dma_start(t0[:], h_in[:, 0:woff + grp * in_bt])

        def w_slice(wp, h, c0, cw):
            return t0[:, wp * 2 * U + h * U + c0:wp * 2 * U + h * U + c0 + cw]

        copy_idx = 0
        for g in range(ngrp):
            if g == 0:
                hx = t0
                base = woff
            else:
                hx = in_pool.tile([128, grp * in_bt], bf16, tag="hx")
                rings[g % 2].dma_start(
                    hx[:], h_in[:, woff + g * grp * in_bt:
                                 woff + (g + 1) * grp * in_bt])
                base = 0
            ot = out_pool.tile([128, grp * out_bt], bf16, tag="ot")
            for q in range(grp):
                for a in (0, 1):            # 0 = real, 1 = imag
                    for (c0, cw) in chunks:
                        ps = ps_mm.tile([128, cw], f32, tag="ps")
                        n_mm = 2 * len(terms)
                        j = 0
                        for (ao, wp) in terms:
                            for h in (0, 1):
                                st = hx[:, base + q * in_bt + (a + ao) * 256
                                        + h * 128:
                                        base + q * in_bt + (a + ao) * 256
                                        + h * 128 + 128]
                                nc.tensor.matmul(
                                    ps[:], st, w_slice(wp, h, c0, cw),
                                    start=(j == 0),
                                    stop=(j == n_mm - 1),
                                )
                                j += 1
                        dst = ot[:, q * out_bt + a * U + c0:
                                 q * out_bt + a * U + c0 + cw]
                        # ~3:2 vector:scalar keeps the PSUM-copy engines
                        # balanced (ACT is a bit slower than DVE).
                        if copy_idx % 5 in (2, 4):
                            nc.scalar.copy(dst, ps[:])
                        else:
                            nc.vector.tensor_copy(dst, ps[:])
                        copy_idx += 1
            # Output rides the opposite ring from this group's input.
            rings[(g + 1) % 2].dma_start(
                out[:, g * grp * out_bt:(g + 1) * grp * out_bt], ot[:])

    nc.compile()
    return nc


def _get_program(U, narr, use_wlo, grp):
    key = (U, narr, use_wlo, grp)
    prog = _cache.get(key)
    if prog is None:
        prog = _build_program(U, narr, use_wlo, grp)
        _cache[key] = prog
    return prog


def _prepare(H_real, H_imag, pilot_loc, alpha, beta):
    """Build (nc, in_maps, assemble) for the full-input problem."""
    H_real = np.ascontiguousarray(np.asarray(H_real, dtype=np.float32))
    H_imag = np.ascontiguousarray(np.asarray(H_imag, dtype=np.float32))
    pilot_loc = np.asarray(pilot_loc, dtype=np.float32)
    alpha = np.asarray(alpha, dtype=np.float32)
    beta = np.asarray(beta, dtype=np.float32)

    W = _interp_matrix(pilot_loc, alpha, beta)
    # Dedupe identical columns: device computes H @ W_u, host expands.
    Wu, inv = np.unique(W, axis=1, return_inverse=True)
    inv = np.asarray(inv).ravel().astype(np.int64)
    U = Wu.shape[1]

    w_hi, w_lo = _bf16_split(Wu)
    use_wlo = bool(np.any(np.asarray(w_lo) != 0))
    # bf16 H alone keeps norm rel err ~1e-3 (gate 2e-2); the lo path is
    # there only for exotic tolerances.
    narr = 2
    grp = _GRP if U <= 512 else 1
    nc = _get_program(U, narr, use_wlo, grp)

    # W head block: [p, wp*2U + h*U + u] = Wpart[h*128 + p, u].
    wparts = [w_hi] + ([w_lo] if use_wlo else [])
    wblk = np.concatenate(
        [np.asarray(wp).reshape(2, 128, U).transpose(1, 0, 2).reshape(128,
                                                                      2 * U)
         for wp in wparts], axis=1)

    hr = H_real.astype(_BF16)
    hi = H_imag.astype(_BF16)

    in_maps = []
    for i in range(_NC):
        # [a, bt, r, h, p] -> [p, bt, a, h, r] so a group's columns are one
        # contiguous dram block per partition line.
        h2 = np.stack([hr[i * _BS:(i + 1) * _BS],
                       hi[i * _BS:(i + 1) * _BS]])
        x = h2.reshape(2, _NBT, _PT, 2, 128).transpose(4, 1, 0, 3, 2)
        m = {
            "hx": np.ascontiguousarray(np.concatenate(
                [wblk, x.reshape(_PT, _NBT * 2 * _P)], axis=1)),
        }
        in_maps.append(m)

    def assemble(results):
        # Per core: [r, bt, a, u] -> [bt*r, a, u]; concat cores; expand u.
        outs = []
        for r in results:
            o = r["out"].reshape(_PT, _NBT, 2, U).transpose(1, 0, 2, 3)
            outs.append(o.reshape(_BS, 2, U))
        ou = np.concatenate(outs, axis=0).astype(np.float32)
        full = np.empty((_B, _NFFT, 2), np.float32)
        full[:, :, 0] = ou[:, 0][:, inv]
        full[:, :, 1] = ou[:, 1][:, inv]
        return full

    return nc, in_maps, assemble


def kernel(H_real, H_imag, pilot_loc, alpha, beta):
    nc, in_maps, assemble = _prepare(H_real, H_imag, pilot_loc, alpha, beta)

    from concourse.bass_utils import run_bass_kernel_spmd

    res = run_bass_kernel_spmd(nc, in_maps, list(range(_NC))).results
    return assemble(res)
